# revision 10
# baseline (speedup 1.0000x reference)
"""Trainium2 Bass kernel for the CoLa MoE-routing module.

Computation (reference semantics):
    att   = q @ Wk.T + bk                  [B, S]
    a     = softmax(top8_mask(att))        [B, S]  (8 nonzero per row)
    out   = sum_s a[:, s] * (x @ V0[s].T @ V1[s].T)   [B, O]

Sharding: expert-parallel over 8 NeuronCores (8 experts each). Each core
receives the full x/q (replicated) and its slice of V0/V1. The expert axis
is rotated per-core in Wk/bk so that every core's local experts are columns
0..7 of its own attention matrix (top-k/softmax are permutation invariant).
Per-core partial outputs are summed on the host.

Shapes are hardcoded for B=256, IN=1024, OUT=1024, SUB=128, S=64, k=8.
"""

import os

import numpy as np

import concourse.bass as bass
import concourse.bacc as bacc
import concourse.mybir as mybir
import concourse.tile as tile
from concourse import bass_utils
from concourse.masks import make_identity

B = 256
IN_F = 1024
OUT_F = 1024
SUB_F = 128
Q_F = 1024
N_SUB = 64
N_ACT = 8
N_CORES = 8
E_LOC = N_SUB // N_CORES  # 8 experts per core

P = 128
BT = B // P  # 2 batch tiles
KC = IN_F // P  # 8 contraction chunks
QC = Q_F // P

F32 = mybir.dt.float32
BF16 = mybir.dt.bfloat16
FP16 = mybir.dt.float16

# weight dtype for x/V0/V1 matmuls
MOE_DTYPE = os.environ.get("MOE_DTYPE", "fp16")
# routing dtype for q/Wk/bias. MUST stay fp32: with fp16 inputs the
# att error (~2e-4) flips the top-8 set on a near-tie row (gap 2.3e-4
# at seed 0), costing 2.2e-2 rel error vs the reference's fp32 top-k.
ROUTE_DTYPE = os.environ.get("MOE_ROUTE_DT", "fp32")
# output partial dtype written to DRAM (host accumulates in fp32)
OUT_DTYPE = os.environ.get("MOE_OUT_DT", "fp16")
# routing-weight broadcast: "flat" = sbuf flatten DMA + gpsimd
# partition_broadcast (expert 0 reads aT8 row 0 directly, skipping the
# flat hop on the critical path). DVE rejects stride-0 partition APs.
MOE_BCAST = os.environ.get("MOE_BCAST", "flat")
# PE warmup matmuls to lift the HAM clock gate before real work arrives
WARMUP_MMS = int(os.environ.get("MOE_WARMUP", "10"))

_DT = {"bf16": BF16, "fp16": FP16, "fp32": F32}


def _bcast_ap(row):
    """[1, N] SBUF AP -> [128, N] partition-broadcast AP (stride-0)."""
    return bass.AP(tensor=row.tensor, offset=row.offset,
                   ap=[[0, P]] + [list(d) for d in row.ap[1:]])


def _build(mode: str):
    wdt = _DT[mode]
    rdt = _DT[ROUTE_DTYPE]
    odt = _DT[OUT_DTYPE]
    nc = bacc.Bacc("TRN2", target_bir_lowering=False, debug=False,
                   num_devices=N_CORES)

    # ---- DRAM I/O (per-core), partition-major so DMAs are contiguous ----
    # wkT carries bk in its last 64 columns (saves a tiny DMA round-trip)
    wkT_d = nc.dram_tensor("wkT", [P, (QC + 1) * N_SUB], rdt,
                           kind="ExternalInput").ap()
    qT_d = nc.dram_tensor("qT", [P, QC, B], rdt, kind="ExternalInput").ap()
    xT_d = nc.dram_tensor("xT", [P, KC, B], wdt, kind="ExternalInput").ap()
    v0t_d = nc.dram_tensor("v0t", [E_LOC, P, KC, SUB_F], wdt,
                           kind="ExternalInput").ap()
    v1t_d = nc.dram_tensor("v1t", [E_LOC, P, OUT_F], wdt,
                           kind="ExternalInput").ap()
    out_d = nc.dram_tensor("out_p", [B, OUT_F], odt, kind="ExternalOutput").ap()

    with tile.TileContext(nc) as tc:
        with (
            tc.tile_pool(name="singles", bufs=1) as singles,
            tc.tile_pool(name="work", bufs=4) as work,
            tc.tile_pool(name="ps_misc", bufs=1, space="PSUM") as ps_misc,
            tc.tile_pool(name="ps_h", bufs=3, space="PSUM") as ps_h,
            tc.tile_pool(name="ps_out", bufs=1, space="PSUM") as ps_out,
        ):
            # ---- constants ----
            ones_sb = singles.tile([1, P], rdt, tag="ones")
            nc.vector.memset(ones_sb, 1.0)
            ident_sb = singles.tile([P, P], F32, tag="ident")
            make_identity(nc, ident_sb)

            # warm up the PE (HAM clock gate needs ~3us of sustained
            # activity to reach full speed) while input DMAs stream
            misc_ps = ps_misc.tile([P, B], F32, tag="misc")
            for _ in range(WARMUP_MMS):
                nc.tensor.matmul(misc_ps[:, 0:P], lhsT=ident_sb,
                                 rhs=ident_sb, start=True, stop=True)

            # ---- input DMAs: two HWDGE queues (sync=SP, scalar=ACT).
            # The DMA bus is shared, transfers serialize at instruction
            # grain across queues, so order == arrival schedule:
            #   SP : wkT | qT c03 | qT c47 | v1 j0..j7
            #   ACT: xT c03 | xT c47 | v0 j0..j7
            # which interleaves to ... q47, v0j0, v1j0, v0j1, v1j1 ...
            wkT_sb = singles.tile([P, (QC + 1) * N_SUB], rdt, tag="wkT")
            nc.sync.dma_start(wkT_sb, wkT_d)
            qT_sb = singles.tile([P, QC, B], rdt, tag="qT")
            nc.sync.dma_start(qT_sb[:, 0:4, :], qT_d[:, 0:4, :])
            nc.sync.dma_start(qT_sb[:, 4:8, :], qT_d[:, 4:8, :])
            xT_sb = singles.tile([P, KC, B], wdt, tag="xT")
            nc.scalar.dma_start(xT_sb[:, 0:4, :], xT_d[:, 0:4, :])
            nc.scalar.dma_start(xT_sb[:, 4:8, :], xT_d[:, 4:8, :])
            v0t_sb = singles.tile([P, E_LOC, KC, SUB_F], wdt, tag="v0t")
            v1t_sb = singles.tile([P, E_LOC, OUT_F], wdt, tag="v1t")
            for j in range(E_LOC):
                nc.scalar.dma_start(v0t_sb[:, j], v0t_d[j])
                nc.sync.dma_start(v1t_sb[:, j], v1t_d[j])

            # ---- routing: att = q @ Wk.T + bk, per batch tile ----
            # att groups borrow the (not-yet-used) out-accumulator banks
            att_ps = [ps_out.tile([P, N_SUB], F32, tag=f"out{bt}",
                                  name=f"att{bt}") for bt in range(BT)]
            for c in range(QC):
                for bt in range(BT):
                    nc.tensor.matmul(
                        att_ps[bt],
                        lhsT=qT_sb[:, c, bt * P:(bt + 1) * P],
                        rhs=wkT_sb[:, c * N_SUB:(c + 1) * N_SUB],
                        start=(c == 0), stop=False,
                    )
            for bt in range(BT):
                # bias: att += 1 (x) bk   (K=1 matmul)
                nc.tensor.matmul(
                    att_ps[bt], lhsT=ones_sb,
                    rhs=wkT_sb[0:1, QC * N_SUB:(QC + 1) * N_SUB],
                    start=False, stop=True)

            # ---- top-8 + softmax (rows = batch), normalized up front ----
            a_bt = []
            for bt in range(BT):
                # |att| <= ~5 so exp(att) cannot overflow: skip the max
                # shift; max8 (for the top-8 threshold) runs in parallel
                m8 = work.tile([P, 8], F32, tag="m8")
                nc.vector.max(out=m8, in_=att_ps[bt])
                e_top = work.tile([P, N_SUB], F32, tag="e_top")
                nc.scalar.activation(e_top, att_ps[bt],
                                     mybir.ActivationFunctionType.Exp)
                # e = (att >= t8) * e_top, denom = row-sum(e), in one op
                e = work.tile([P, N_SUB], F32, tag="e")
                denom = work.tile([P, 1], F32, tag="denom")
                nc.vector.scalar_tensor_tensor(
                    e, att_ps[bt], m8[:, 7:8], e_top,
                    op0=mybir.AluOpType.is_ge, op1=mybir.AluOpType.mult,
                    accum_out=denom)
                recip = work.tile([P, 1], F32, tag="recip")
                nc.vector.reciprocal(recip, denom)
                a = work.tile([P, N_SUB], F32, tag="a")
                nc.vector.tensor_scalar_mul(a, e, recip)
                a_bt.append(a)

            # ---- expert loop, interleaved so PE never waits on routing:
            # m1 j0 | trs | m1 j1 | m2 j0 | m1 j2 | m2 j1 | ... | m2 j7
            aT8_sb = singles.tile([E_LOC, B], F32, tag="aT8")
            flat_sb = singles.tile([1, E_LOC * B], F32, tag="flat")
            out_ps = [ps_out.tile([P, OUT_F], F32, tag=f"out{bt}",
                                  name=f"out_ps{bt}")
                      for bt in range(BT)]
            h_ps = [None] * E_LOC
            hs_sb = [None] * E_LOC

            def m1(j):
                h_ps[j] = ps_h.tile([P, B], F32, tag="h", name=f"h{j}")
                for c in range(KC):
                    nc.tensor.matmul(h_ps[j], lhsT=v0t_sb[:, j, c, :],
                                     rhs=xT_sb[:, c, :],
                                     start=(c == 0), stop=(c == KC - 1))

            def hs(j):
                # hs = h * a[:, j]  (PSUM x broadcast row -> SBUF, cast)
                hs_sb[j] = work.tile([P, B], wdt, tag="hs", name=f"hs{j}")
                abc = work.tile([P, B], F32, tag="abc", name=f"abc{j}")
                src = aT8_sb[0:1, :] if j == 0 \
                    else flat_sb[:, j * B:(j + 1) * B]
                nc.gpsimd.partition_broadcast(abc, src)
                nc.vector.tensor_tensor(hs_sb[j], h_ps[j], abc,
                                        mybir.AluOpType.mult)

            def m2(j):
                for bt in range(BT):
                    for nh in range(2):
                        nc.tensor.matmul(
                            out_ps[bt][:, nh * 512:(nh + 1) * 512],
                            lhsT=hs_sb[j][:, bt * P:(bt + 1) * P],
                            rhs=v1t_sb[:, j, nh * 512:(nh + 1) * 512],
                            start=(j == 0), stop=(j == E_LOC - 1),
                        )

            m1(0)
            # one transpose per batch tile into a shared PSUM tile, then a
            # single copy: aT8[j, bt*128+b] = a_bt[bt][b, j]
            for bt in range(BT):
                nc.tensor.transpose(
                    misc_ps[0:E_LOC, bt * P:(bt + 1) * P],
                    a_bt[bt][:, 0:E_LOC], ident_sb)
            nc.vector.tensor_copy(aT8_sb, misc_ps[0:E_LOC, :])
            # flatten the 8 expert rows into one partition so
            # partition_broadcast can address each. On the otherwise-idle
            # Pool SWDGE queue: the SP/ACT queues are deep in expert-weight
            # DMAs at this point and would delay it by ~8us.
            nc.gpsimd.dma_start(flat_sb, aT8_sb)
            hs(0)
            m1(1)
            for j in range(E_LOC - 1):
                m2(j)
                if j + 2 < E_LOC:
                    m1(j + 2)
                    hs(j + 1)
                else:
                    hs(j + 1)
            m2(E_LOC - 1)

            # ---- write out (copies split DVE/ACT, DMAs on both rings) ----
            for bt in range(BT):
                for nh in range(2):
                    o_sb = work.tile([P, 512], odt, tag="o_sb")
                    src = out_ps[bt][:, nh * 512:(nh + 1) * 512]
                    if nh == 0:
                        nc.vector.tensor_copy(o_sb, src)
                    else:
                        nc.scalar.activation(
                            o_sb, src, mybir.ActivationFunctionType.Copy)
                    eng = nc.sync if bt == 0 else nc.scalar
                    eng.dma_start(
                        out_d[bt * P:(bt + 1) * P, nh * 512:(nh + 1) * 512],
                        o_sb)

    nc.compile()
    return nc


_CACHE = {}


def _get_nc(mode: str):
    if mode not in _CACHE:
        _CACHE[mode] = _build(mode)
    return _CACHE[mode]


def _pmajor(aT):
    """[D, N] (D = C*128, row-major) -> [128, C, N] partition-major."""
    d, n = aT.shape
    return np.ascontiguousarray(
        aT.reshape(d // P, P, n).transpose(1, 0, 2))


def _np_dt(name):
    import ml_dtypes
    return {"bf16": ml_dtypes.bfloat16, "fp16": np.float16,
            "fp32": np.float32}[name]


def _prep_in_maps(x, q, Wk, bk, V0, V1, mode: str):
    wdt = _np_dt(mode)
    rdt = _np_dt(ROUTE_DTYPE)

    qT = _pmajor(q.T.astype(np.float32)).astype(rdt)      # [128, QC, B]
    xT = _pmajor(x.T).astype(wdt)                         # [128, KC, B]
    # all-expert partition-major views, shared across the per-core loop:
    # v0pm[s] = [P, KC, SUB_F] of V0[s].T;  v1pm[s] = [SUB_F, OUT_F]
    v0pm = V0.transpose(0, 2, 1).reshape(
        N_SUB, KC, P, SUB_F).transpose(0, 2, 1, 3)
    v1pm = V1.transpose(0, 2, 1)
    in_maps = []
    for c in range(N_CORES):
        rot = np.roll(np.arange(N_SUB), -E_LOC * c)
        wk_pm = _pmajor(Wk[rot].T.astype(np.float32))     # [128, QC, S]
        bk_bc = np.broadcast_to(bk[rot].astype(np.float32), (P, N_SUB))
        wkT = np.ascontiguousarray(np.concatenate(
            [wk_pm.reshape(P, QC * N_SUB), bk_bc], axis=1)).astype(rdt)
        base = E_LOC * c
        # v0t: [E_LOC, P, KC, SUB_F]; v1t: [E_LOC, P, OUT_F]
        v0t = v0pm[base:base + E_LOC].astype(wdt, order="C")
        v1t = v1pm[base:base + E_LOC].astype(wdt, order="C")
        in_maps.append({
            "qT": qT, "wkT": wkT, "xT": xT,
            "v0t": v0t, "v1t": v1t,
        })
    return in_maps


def run(inputs: dict, mode: str = MOE_DTYPE, trace: bool = False):
    """Run the distributed kernel; returns (out [B, OUT_F] fp32, results)."""
    nc = _get_nc(mode)
    in_maps = _prep_in_maps(**inputs, mode=mode)
    res = bass_utils.run_bass_kernel_spmd(
        nc, in_maps, core_ids=list(range(N_CORES)), trace=trace,
    )
    out = np.zeros((B, OUT_F), np.float32)
    for c in range(N_CORES):
        out += np.asarray(res.results[c]["out_p"], dtype=np.float32)
    return out, res


def kernel(x, q, Wk, bk, V0, V1):
    x = np.asarray(x, np.float32)
    q = np.asarray(q, np.float32)
    Wk = np.asarray(Wk, np.float32)
    bk = np.asarray(bk, np.float32)
    V0 = np.asarray(V0, np.float32)
    V1 = np.asarray(V1, np.float32)
    out, _ = run(dict(x=x, q=q, Wk=Wk, bk=bk, V0=V0, V1=V1))
    return out


# revision 22
# speedup vs baseline: 1.2006x; 1.2006x over previous
"""Trainium2 Bass kernel for the CoLa MoE-routing module.

Computation (reference semantics):
    att   = q @ Wk.T + bk                  [B, S]
    a     = softmax(top8_mask(att))        [B, S]  (8 nonzero per row)
    out   = sum_s a[:, s] * (x @ V0[s].T @ V1[s].T)   [B, O]

Sharding: expert-parallel over 8 NeuronCores (8 experts each). Each core
receives the full x/q (replicated) and its slice of V0/V1. The expert axis
is rotated per-core in Wk/bk so that every core's local experts are columns
0..7 of its own attention matrix (top-k/softmax are permutation invariant).
Per-core partial outputs are summed on the host.

Dtypes (error budget is 2e-2 fro):
  - q/Wk fp32: att err ~2e-4 with fp16 flips the top-8 set on a near-tie
    row (gap 2.3e-4 at seed 0) costing 2.2e-2 alone.
  - x/V0/hs fp16, V1 float8_e3m4 (measured 1.35e-2 total), out fp16.

The two HWDGE queues (SP=sync, ACT=scalar) share ~190GB/s of HBM and are
FIFO per queue, so the per-queue issue order is a just-in-time schedule:
v0 early (feeds m1 j), q mid (routing only gates the hs/m2 side), v1
late, with the final arrivals being tensors with the shortest dependent
chain.

Shapes are hardcoded for B=256, IN=1024, OUT=1024, SUB=128, S=64, k=8.
"""

import os

import numpy as np

import concourse.bass as bass
import concourse.bacc as bacc
import concourse.mybir as mybir
import concourse.tile as tile
from concourse import bass_utils
from concourse.masks import make_identity

B = 256
IN_F = 1024
OUT_F = 1024
SUB_F = 128
Q_F = 1024
N_SUB = 64
N_ACT = 8
N_CORES = 8
E_LOC = N_SUB // N_CORES  # 8 experts per core

P = 128
BT = B // P  # 2 batch tiles
KC = IN_F // P  # 8 contraction chunks
QC = Q_F // P

F32 = mybir.dt.float32
FP16 = mybir.dt.float16
F8E3 = mybir.dt.float8e3

MOE_DTYPE = os.environ.get("MOE_DTYPE", "fp16")      # x/V0/hs dtype
MOE_V1_DT = os.environ.get("MOE_V1_DT", "e3m4")      # V1 dtype
OUT_DTYPE = os.environ.get("MOE_OUT_DT", "fp16")     # output partials
WARMUP_MMS = int(os.environ.get("MOE_WARMUP", "12"))

_DT = {"fp16": FP16, "fp32": F32, "e3m4": F8E3}


def _build(mode: str):
    wdt = _DT[mode]
    v1dt = _DT[MOE_V1_DT]
    odt = _DT[OUT_DTYPE]
    nc = bacc.Bacc("TRN2", target_bir_lowering=False, debug=False,
                   num_devices=N_CORES)

    # ---- DRAM I/O (per-core), partition-major so DMAs are contiguous ----
    # wkT carries bk in its last 64 columns
    wkT_d = nc.dram_tensor("wkT", [P, (QC + 1) * N_SUB], F32,
                           kind="ExternalInput").ap()
    qT_d = nc.dram_tensor("qT", [P, QC, B], F32, kind="ExternalInput").ap()
    xT_d = nc.dram_tensor("xT", [P, KC, B], wdt, kind="ExternalInput").ap()
    v0t_d = nc.dram_tensor("v0t", [E_LOC, P, KC, SUB_F], wdt,
                           kind="ExternalInput").ap()
    # expert pairs g = (2g, 2g+1) partition-major so a pair DMA is one
    # contiguous 2KB line per partition
    v1t_d = nc.dram_tensor("v1t", [E_LOC // 2, P, 2, OUT_F], v1dt,
                           kind="ExternalInput").ap()
    out_d = nc.dram_tensor("out_p", [B, OUT_F], odt, kind="ExternalOutput").ap()

    with tile.TileContext(nc) as tc:
        with (
            tc.tile_pool(name="singles", bufs=1) as singles,
            tc.tile_pool(name="work", bufs=4) as work,
            tc.tile_pool(name="ps_misc", bufs=1, space="PSUM") as ps_misc,
            tc.tile_pool(name="ps_h", bufs=3, space="PSUM") as ps_h,
            tc.tile_pool(name="ps_out", bufs=1, space="PSUM") as ps_out,
        ):
            # ---- constants ----
            ones_sb = singles.tile([1, P], F32, tag="ones")
            nc.vector.memset(ones_sb, 1.0)
            ident_sb = singles.tile([P, P], F32, tag="ident")
            make_identity(nc, ident_sb)
            ident16_sb = singles.tile([P, P], FP16, tag="ident16")
            nc.vector.tensor_copy(ident16_sb, ident_sb)

            # warm up the PE while input DMAs stream (f32 idents burn
            # 4 cycles/row, bridging the DMA head without real work)
            misc_ps = ps_misc.tile([P, B], F32, tag="misc", name="warm")
            for _ in range(WARMUP_MMS):
                nc.tensor.matmul(misc_ps[:, 0:P], lhsT=ident_sb,
                                 rhs=ident_sb, start=True, stop=True)

            # ---- input DMAs (order == just-in-time arrival schedule) ----
            wkT_sb = singles.tile([P, (QC + 1) * N_SUB], F32, tag="wkT")
            qT_sb = [singles.tile([P, 4, B], F32, tag=f"qT{h}",
                                  name=f"qT{h}") for h in range(2)]
            xT_sb = singles.tile([P, KC, B], wdt, tag="xT")
            v0_sb = [singles.tile([P, KC, SUB_F], wdt, tag=f"v0_{j}",
                                  name=f"v0_{j}") for j in range(E_LOC)]
            # v1 grouped to keep DMA lines >= 1KB: j01 pair, j23, j45, j67
            v1_sb = [singles.tile([P, 2, OUT_F], v1dt, tag=f"v1_{g}",
                                  name=f"v1_{g}") for g in range(4)]

            # SP queue: wk | v0j0 | q03 | v0j3 | v1j01 | v0j5 | v0j6 | v1j45
            nc.sync.dma_start(wkT_sb, wkT_d)
            nc.sync.dma_start(v0_sb[0], v0t_d[0])
            nc.sync.dma_start(qT_sb[0], qT_d[:, 0:4, :])
            nc.sync.dma_start(v0_sb[3], v0t_d[3])
            nc.sync.dma_start(v1_sb[0], v1t_d[0])
            nc.sync.dma_start(v0_sb[5], v0t_d[5])
            nc.sync.dma_start(v0_sb[6], v0t_d[6])
            nc.sync.dma_start(v1_sb[2], v1t_d[2])
            # ACT queue: x | v0j1 | q47 | v0j2 | v0j4 | v1j23 | v0j7 | v1j67
            nc.scalar.dma_start(xT_sb, xT_d)
            nc.scalar.dma_start(v0_sb[1], v0t_d[1])
            nc.scalar.dma_start(qT_sb[1], qT_d[:, 4:8, :])
            nc.scalar.dma_start(v0_sb[2], v0t_d[2])
            nc.scalar.dma_start(v0_sb[4], v0t_d[4])
            nc.scalar.dma_start(v1_sb[1], v1t_d[1])
            nc.scalar.dma_start(v0_sb[7], v0t_d[7])
            nc.scalar.dma_start(v1_sb[3], v1t_d[3])

            # ---- routing: att = q @ Wk.T + bk, per batch tile ----
            # att groups borrow the (not-yet-used) out-accumulator banks
            att_ps = [ps_out.tile([P, N_SUB], F32, tag=f"out{bt}",
                                  name=f"att{bt}") for bt in range(BT)]

            def att_matmuls():
                for c in range(QC):
                    for bt in range(BT):
                        nc.tensor.matmul(
                            att_ps[bt],
                            lhsT=qT_sb[c // 4][:, c % 4, bt * P:(bt + 1) * P],
                            rhs=wkT_sb[:, c * N_SUB:(c + 1) * N_SUB],
                            start=(c == 0), stop=False,
                        )
                for bt in range(BT):
                    nc.tensor.matmul(
                        att_ps[bt], lhsT=ones_sb,
                        rhs=wkT_sb[0:1, QC * N_SUB:(QC + 1) * N_SUB],
                        start=False, stop=True)

            def routing_vector(bt, a16):
                # |att| <= ~5 so exp(att) cannot overflow: skip the max
                # shift; max8 (for the top-8 threshold) runs on DVE while
                # exp runs on ACT
                m8 = work.tile([P, 8], F32, tag="m8")
                nc.vector.max(out=m8, in_=att_ps[bt])
                e_top = work.tile([P, N_SUB], F32, tag="e_top")
                nc.scalar.activation(e_top, att_ps[bt],
                                     mybir.ActivationFunctionType.Exp)
                # e = (att >= t8) * e_top, denom = row-sum(e), in one op
                e = work.tile([P, N_SUB], F32, tag="e")
                denom = work.tile([P, 1], F32, tag="denom")
                nc.vector.scalar_tensor_tensor(
                    e, att_ps[bt], m8[:, 7:8], e_top,
                    op0=mybir.AluOpType.is_ge, op1=mybir.AluOpType.mult,
                    accum_out=denom)
                recip = work.tile([P, 1], F32, tag="recip")
                nc.vector.reciprocal(recip, denom)
                # normalized fp16 weights for the local experts only
                nc.vector.tensor_scalar_mul(a16, e[:, 0:E_LOC], recip)

            # ---- expert loop ----
            out_ps = [ps_out.tile([P, OUT_F], F32, tag=f"out{bt}",
                                  name=f"out_ps{bt}")
                      for bt in range(BT)]
            h_ps = [None] * E_LOC
            hs_sb = [None] * E_LOC
            a16_bt = [singles.tile([P, E_LOC], FP16, tag=f"a16_{bt}",
                                   name=f"a16_{bt}") for bt in range(BT)]
            # flat routing rows: flat[g][0, (j%4)*256 + bt*128 + p] =
            # a16_bt[bt][p, j]; built by 16 single-column fp16 PE
            # transposes into a partition-0 PSUM row (misc bank reused),
            # copied to SBUF per half. No DMA on the routing path.
            flat_sb = [singles.tile([1, 4 * B], FP16, tag=f"flat{g}",
                                    name=f"flat{g}") for g in range(2)]

            def flat_half(g):
                fl_ps = ps_misc.tile([1, 4 * B], FP16, tag="misc",
                                     name=f"flat_ps{g}")
                for j in range(4):
                    for bt in range(BT):
                        nc.tensor.transpose(
                            fl_ps[0:1, j * B + bt * P:j * B + (bt + 1) * P],
                            a16_bt[bt][:, 4 * g + j:4 * g + j + 1],
                            ident16_sb)
                nc.vector.tensor_copy(flat_sb[g], fl_ps)

            def m1(j):
                h_ps[j] = ps_h.tile([P, B], F32, tag="h", name=f"h{j}")
                for c in range(KC):
                    nc.tensor.matmul(h_ps[j], lhsT=v0_sb[j][:, c, :],
                                     rhs=xT_sb[:, c, :],
                                     start=(c == 0), stop=(c == KC - 1))

            def hs(j):
                # hs = h * a[:, j]  (PSUM x broadcast row -> SBUF, cast)
                hs_sb[j] = work.tile([P, B], wdt, tag="hs", name=f"hs{j}")
                abc = work.tile([P, B], FP16, tag="abc", name=f"abc{j}")
                nc.gpsimd.partition_broadcast(
                    abc, flat_sb[j // 4][:, (j % 4) * B:(j % 4 + 1) * B])
                nc.vector.tensor_tensor(hs_sb[j], h_ps[j], abc,
                                        mybir.AluOpType.mult)

            def m2(j):
                for bt in range(BT):
                    for nh in range(2):
                        nc.tensor.matmul(
                            out_ps[bt][:, nh * 512:(nh + 1) * 512],
                            lhsT=hs_sb[j][:, bt * P:(bt + 1) * P],
                            rhs=v1_sb[j // 2][:, j % 2,
                                              nh * 512:(nh + 1) * 512],
                            start=(j == 0), stop=(j == E_LOC - 1),
                        )

            # PE order keeps the engine fed while routing inputs stream:
            # m1 j0/j1 run before att (v0 arrives first), ident fillers
            # bridge arrival gaps (keeping the PE clock gate open), the
            # a-transposes slot between expert matmuls, and each m2 pair
            # rides its v1 arrival.
            m1(0)
            m1(1)
            for _ in range(16):
                nc.tensor.matmul(misc_ps[:, 0:P], lhsT=ident_sb,
                                 rhs=ident_sb, start=True, stop=True)
            att_matmuls()
            for bt in range(BT):
                routing_vector(bt, a16_bt[bt])
            m1(2)
            flat_half(0)
            flat_half(1)
            hs(0)
            hs(1)
            hs(2)
            m2(0)
            m1(3)
            m2(1)
            m1(4)
            hs(3)
            m2(2)
            m1(5)
            hs(4)
            m2(3)
            m1(6)
            hs(5)
            m2(4)
            m1(7)
            hs(6)
            m2(5)
            hs(7)
            m2(6)
            m2(7)

            # ---- write out (copies split DVE/ACT, DMAs on both rings) ----
            for bt in range(BT):
                for nh in range(2):
                    o_sb = work.tile([P, 512], odt, tag="o_sb")
                    src = out_ps[bt][:, nh * 512:(nh + 1) * 512]
                    if nh == 0:
                        nc.vector.tensor_copy(o_sb, src)
                    else:
                        nc.scalar.activation(
                            o_sb, src, mybir.ActivationFunctionType.Copy)
                    eng = nc.sync if bt == 0 else nc.scalar
                    eng.dma_start(
                        out_d[bt * P:(bt + 1) * P, nh * 512:(nh + 1) * 512],
                        o_sb)

    nc.compile()
    return nc


_CACHE = {}


def _get_nc(mode: str):
    if mode not in _CACHE:
        _CACHE[mode] = _build(mode)
    return _CACHE[mode]


def _pmajor(aT):
    """[D, N] (D = C*128, row-major) -> [128, C, N] partition-major."""
    d, n = aT.shape
    return np.ascontiguousarray(
        aT.reshape(d // P, P, n).transpose(1, 0, 2))


def _np_dt(name):
    import ml_dtypes
    return {"fp16": np.float16, "fp32": np.float32,
            "e3m4": ml_dtypes.float8_e3m4}[name]


def _prep_in_maps(x, q, Wk, bk, V0, V1, mode: str):
    wdt = _np_dt(mode)
    v1dt = _np_dt(MOE_V1_DT)

    qT = _pmajor(q.T.astype(np.float32))                  # [128, QC, B]
    xT = _pmajor(x.T).astype(wdt)                         # [128, KC, B]
    # all-expert partition-major views, shared across the per-core loop:
    # v0pm[s] = [P, KC, SUB_F] of V0[s].T;  v1pm[s] = [SUB_F, OUT_F]
    v0pm = V0.transpose(0, 2, 1).reshape(
        N_SUB, KC, P, SUB_F).transpose(0, 2, 1, 3)
    v1pm = V1.transpose(0, 2, 1)
    in_maps = []
    for c in range(N_CORES):
        rot = np.roll(np.arange(N_SUB), -E_LOC * c)
        wk_pm = _pmajor(Wk[rot].T.astype(np.float32))     # [128, QC, S]
        bk_bc = np.broadcast_to(bk[rot].astype(np.float32), (P, N_SUB))
        wkT = np.ascontiguousarray(np.concatenate(
            [wk_pm.reshape(P, QC * N_SUB), bk_bc], axis=1))
        base = E_LOC * c
        # v0t: [E_LOC, P, KC, SUB_F]; v1t: [E_LOC//2, P, 2, OUT_F]
        v0t = v0pm[base:base + E_LOC].astype(wdt, order="C")
        v1t = v1pm[base:base + E_LOC].reshape(
            E_LOC // 2, 2, P, OUT_F).transpose(0, 2, 1, 3).astype(
            v1dt, order="C")
        in_maps.append({
            "qT": qT, "wkT": wkT, "xT": xT,
            "v0t": v0t, "v1t": v1t,
        })
    return in_maps


def run(inputs: dict, mode: str = MOE_DTYPE, trace: bool = False):
    """Run the distributed kernel; returns (out [B, OUT_F] fp32, results)."""
    nc = _get_nc(mode)
    in_maps = _prep_in_maps(**inputs, mode=mode)
    res = bass_utils.run_bass_kernel_spmd(
        nc, in_maps, core_ids=list(range(N_CORES)), trace=trace,
    )
    out = np.zeros((B, OUT_F), np.float32)
    for c in range(N_CORES):
        out += np.asarray(res.results[c]["out_p"], dtype=np.float32)
    return out, res


def kernel(x, q, Wk, bk, V0, V1):
    x = np.asarray(x, np.float32)
    q = np.asarray(q, np.float32)
    Wk = np.asarray(Wk, np.float32)
    bk = np.asarray(bk, np.float32)
    V0 = np.asarray(V0, np.float32)
    V1 = np.asarray(V1, np.float32)
    out, _ = run(dict(x=x, q=q, Wk=Wk, bk=bk, V0=V0, V1=V1))
    return out
